# revision 5
# baseline (speedup 1.0000x reference)
"""Trainium2 Bass kernel for nn_CustomLoss (exp(-pairwise_distance) weighted loss).

Strategy (data-parallel over 8 NeuronCores):
  - Shard the batch dim B=16384 across 8 cores (2048 rows each).
  - Each core streams its [2048, 4096] f32 train shard from HBM in row-tiles
    of [128, 4096]; DVE computes diff = (test+eps) - x, ACT computes
    Square(diff) with a fused free-dim accumulation -> per-row sum of squared
    differences. This is the memory-bound part (268 MB total HBM read).
  - (test+eps) is loaded once as a single 16 KB row and broadcast across the
    128 partitions on-chip (gpsimd partition_broadcast) to keep the DMA
    fabric free for the train stream.
  - The last row-tile is processed in four [128, 1024] column-quarters so the
    compute exposed after the final DMA byte is ~2 us instead of ~8 us.
  - The tiny [B] tail (sqrt, exp, median threshold, normalized weighted sum)
    runs on host, faithfully emulating the reference's fp32/XLA semantics
    (XLA's f32 exp flushes subnormal outputs to zero).
"""

import numpy as np

B = 16384
F = 4096
NCORES = 8
ROWS = B // NCORES  # rows per core
P = 128  # SBUF partitions
TILES = ROWS // P  # row-tiles per core
NQ = 4  # column-quarters for the last row-tile
QF = F // NQ
OUT_COLS = TILES - 1 + NQ
EPS = 1e-6

_cached_nc = None
_last_results = None
TRACE = False


def _build_bass():
    import concourse.bacc as bacc
    import concourse.tile as tile
    from concourse import mybir

    f32 = mybir.dt.float32
    nc = bacc.Bacc("TRN2", target_bir_lowering=False, enable_partition_id=False)
    train = nc.dram_tensor("train", [ROWS, F], f32, kind="ExternalInput")
    avec = nc.dram_tensor("avec", [1, F], f32, kind="ExternalInput")
    out = nc.dram_tensor("sumsq", [P, OUT_COLS], f32, kind="ExternalOutput")

    # Column split of the per-tile subtract: DVE takes [0, VCOLS), gpsimd
    # takes [VCOLS, F) so neither engine paces below the DMA stream.
    VCOLS = 2560

    with tile.TileContext(nc) as tc:
        with (
            tc.tile_pool(name="singles", bufs=1) as singles,
            tc.tile_pool(name="loads", bufs=5) as loads,
            tc.tile_pool(name="diffs", bufs=3) as diffs,
            tc.tile_pool(name="psum", bufs=1, space="PSUM") as psum,
        ):
            # (test + eps): one 16 KB row DMA, rank-1 PE matmul broadcast
            # (ones[1,128].T @ a_row[1,F]) into PSUM, then one ACT copy to
            # SBUF. Keeps the DMA fabric free for the train stream.
            a_row = singles.tile([1, F], f32)
            nc.sync.dma_start(out=a_row[:, :], in_=avec[:, :])
            ones = singles.tile([1, P], f32)
            nc.vector.memset(ones, 1.0)
            a_ps = psum.tile([P, F], f32)
            BANK = 512
            for b_ in range(F // BANK):
                sl = slice(b_ * BANK, (b_ + 1) * BANK)
                nc.tensor.matmul(a_ps[:, sl], ones[:, :], a_row[:, sl])
            a_sb = singles.tile([P, F], f32)
            nc.scalar.copy(a_sb[:, :], a_ps[:, :])

            zeros = singles.tile([P, 1], f32)
            nc.vector.memset(zeros, 0.0)

            acc = singles.tile([P, OUT_COLS], f32)
            tr = train[:, :].rearrange("(t p) f -> t p f", p=P)
            col = 0
            for t in range(TILES):
                if t < TILES - 1:
                    spans = [(0, F)]
                else:
                    spans = [(q * QF, QF) for q in range(NQ)]
                for f0, fw in spans:
                    x = loads.tile([P, fw], f32, tag="x")
                    nc.sync.dma_start(out=x[:, :], in_=tr[t, :, f0 : f0 + fw])
                    d = diffs.tile([P, fw], f32, tag="d")
                    if fw == F:
                        nc.vector.tensor_sub(
                            d[:, :VCOLS], a_sb[:, :VCOLS], x[:, :VCOLS]
                        )
                        nc.gpsimd.tensor_sub(
                            d[:, VCOLS:], a_sb[:, VCOLS:], x[:, VCOLS:]
                        )
                    else:
                        nc.vector.tensor_sub(
                            d[:, :], a_sb[:, f0 : f0 + fw], x[:, :]
                        )
                    nc.scalar.activation(
                        out=d[:, :],
                        in_=d[:, :],
                        func=mybir.ActivationFunctionType.Square,
                        bias=zeros[:, :],
                        accum_out=acc[:, col : col + 1],
                    )
                    col += 1
            nc.sync.dma_start(out=out[:, :], in_=acc[:, :])
    nc.finalize()
    return nc


def _device_sumsq(train_data: np.ndarray, test_data: np.ndarray) -> np.ndarray:
    from concourse import bass_utils

    global _cached_nc, _last_results
    if _cached_nc is None:
        _cached_nc = _build_bass()
    a = (test_data.reshape(1, F).astype(np.float32) + np.float32(EPS)).astype(
        np.float32
    )
    in_maps = [
        {
            "train": np.ascontiguousarray(
                train_data[c * ROWS : (c + 1) * ROWS], dtype=np.float32
            ),
            "avec": a,
        }
        for c in range(NCORES)
    ]
    res = bass_utils.run_bass_kernel_spmd(
        _cached_nc, in_maps, core_ids=list(range(NCORES)), trace=TRACE
    )
    _last_results = res
    shards = []
    for r in res.results:
        part = r["sumsq"]  # [128, OUT_COLS]
        full = part[:, : TILES - 1].T.reshape(-1)  # rows t*128+p, t<TILES-1
        last = np.sum(part[:, TILES - 1 :], axis=1, dtype=np.float32)
        shards.append(np.concatenate([full, last]))
    return np.concatenate(shards)


def kernel(pred_batch, target_batch, train_data, test_data):
    sumsq = _device_sumsq(
        np.asarray(train_data, dtype=np.float32),
        np.asarray(test_data, dtype=np.float32),
    )
    dist = np.sqrt(sumsq.astype(np.float32))
    with np.errstate(divide="ignore", invalid="ignore", under="ignore"):
        diag = np.exp(-dist).astype(np.float32)
        # The reference runs under XLA, whose f32 exp flushes subnormal
        # outputs to zero; match that.
        diag = np.where(diag < np.float32(1.1754944e-38), np.float32(0.0), diag)
        med = np.sort(diag)[(B - 1) // 2]
        diag = np.where(diag < med, np.float32(0.0), diag).astype(np.float32)
        s = np.float32(np.sum(diag, dtype=np.float32))
        w = diag / s
        residual = (
            np.asarray(target_batch, dtype=np.float32)
            - np.asarray(pred_batch, dtype=np.float32)
        )[:, 0]
        loss = np.float32(np.sum(w * residual * residual, dtype=np.float32))
    return np.asarray(loss, dtype=np.float32)


# revision 6
# speedup vs baseline: 1.2572x; 1.2572x over previous
"""Trainium2 Bass kernel for nn_CustomLoss (exp(-pairwise_distance) weighted loss).

Strategy (data-parallel over 8 NeuronCores):
  - Shard the batch dim B=16384 across 8 cores (2048 rows each).
  - Each core streams its [2048, 4096] f32 train shard from HBM in row-tiles
    of [128, 4096]; DVE computes diff = (test+eps) - x, ACT computes
    Square(diff) with a fused free-dim accumulation -> per-row sum of squared
    differences. This is the memory-bound part (268 MB total HBM read).
  - (test+eps) is loaded once as a single 16 KB row and broadcast across the
    128 partitions on-chip (gpsimd partition_broadcast) to keep the DMA
    fabric free for the train stream.
  - The last row-tile is processed in four [128, 1024] column-quarters so the
    compute exposed after the final DMA byte is ~2 us instead of ~8 us.
  - The tiny [B] tail (sqrt, exp, median threshold, normalized weighted sum)
    runs on host, faithfully emulating the reference's fp32/XLA semantics
    (XLA's f32 exp flushes subnormal outputs to zero).
"""

import numpy as np

B = 16384
F = 4096
NCORES = 8
ROWS = B // NCORES  # rows per core
P = 128  # SBUF partitions
TILES = ROWS // P  # row-tiles per core
NQ = 4  # column-quarters for the last row-tile
QF = F // NQ
OUT_COLS = TILES - 1 + NQ
EPS = 1e-6

_cached_nc = None
_last_results = None
TRACE = False


def _build_bass():
    import concourse.bacc as bacc
    import concourse.tile as tile
    from concourse import mybir

    f32 = mybir.dt.float32
    nc = bacc.Bacc("TRN2", target_bir_lowering=False, enable_partition_id=False)
    train = nc.dram_tensor("train", [ROWS, F], f32, kind="ExternalInput")
    avec = nc.dram_tensor("avec", [1, F], f32, kind="ExternalInput")
    out = nc.dram_tensor("sumsq", [P, OUT_COLS], f32, kind="ExternalOutput")

    with tile.TileContext(nc) as tc:
        with (
            tc.tile_pool(name="singles", bufs=1) as singles,
            tc.tile_pool(name="loads", bufs=7) as loads,
            tc.tile_pool(name="diffs", bufs=2) as diffs,
        ):
            # (test + eps): one 16 KB row DMA, then broadcast across
            # partitions on-chip (gpsimd ucode op; the DMA fabric stays
            # free for the train stream). Its ~15 us latency is hidden by
            # the 7-deep loads pool.
            a_row = singles.tile([1, F], f32)
            nc.sync.dma_start(out=a_row[:, :], in_=avec[:, :])
            a_sb = singles.tile([P, F], f32)
            nc.gpsimd.partition_broadcast(a_sb[:, :], a_row[:, :])

            zeros = singles.tile([P, 1], f32)
            nc.vector.memset(zeros, 0.0)

            acc = singles.tile([P, OUT_COLS], f32)
            tr = train[:, :].rearrange("(t p) f -> t p f", p=P)
            col = 0
            for t in range(TILES):
                if t < TILES - 1:
                    spans = [(0, F)]
                else:
                    spans = [(q * QF, QF) for q in range(NQ)]
                for f0, fw in spans:
                    x = loads.tile([P, fw], f32, tag="x")
                    nc.sync.dma_start(out=x[:, :], in_=tr[t, :, f0 : f0 + fw])
                    d = diffs.tile([P, fw], f32, tag="d")
                    nc.vector.tensor_sub(d[:, :], a_sb[:, f0 : f0 + fw], x[:, :])
                    nc.scalar.activation(
                        out=d[:, :],
                        in_=d[:, :],
                        func=mybir.ActivationFunctionType.Square,
                        bias=zeros[:, :],
                        accum_out=acc[:, col : col + 1],
                    )
                    col += 1
            nc.sync.dma_start(out=out[:, :], in_=acc[:, :])
    nc.finalize()
    return nc


def _device_sumsq(train_data: np.ndarray, test_data: np.ndarray) -> np.ndarray:
    from concourse import bass_utils

    global _cached_nc, _last_results
    if _cached_nc is None:
        _cached_nc = _build_bass()
    a = (test_data.reshape(1, F).astype(np.float32) + np.float32(EPS)).astype(
        np.float32
    )
    in_maps = [
        {
            "train": np.ascontiguousarray(
                train_data[c * ROWS : (c + 1) * ROWS], dtype=np.float32
            ),
            "avec": a,
        }
        for c in range(NCORES)
    ]
    res = bass_utils.run_bass_kernel_spmd(
        _cached_nc, in_maps, core_ids=list(range(NCORES)), trace=TRACE
    )
    _last_results = res
    shards = []
    for r in res.results:
        part = r["sumsq"]  # [128, OUT_COLS]
        full = part[:, : TILES - 1].T.reshape(-1)  # rows t*128+p, t<TILES-1
        last = np.sum(part[:, TILES - 1 :], axis=1, dtype=np.float32)
        shards.append(np.concatenate([full, last]))
    return np.concatenate(shards)


def kernel(pred_batch, target_batch, train_data, test_data):
    sumsq = _device_sumsq(
        np.asarray(train_data, dtype=np.float32),
        np.asarray(test_data, dtype=np.float32),
    )
    dist = np.sqrt(sumsq.astype(np.float32))
    with np.errstate(divide="ignore", invalid="ignore", under="ignore"):
        diag = np.exp(-dist).astype(np.float32)
        # The reference runs under XLA, whose f32 exp flushes subnormal
        # outputs to zero; match that.
        diag = np.where(diag < np.float32(1.1754944e-38), np.float32(0.0), diag)
        med = np.sort(diag)[(B - 1) // 2]
        diag = np.where(diag < med, np.float32(0.0), diag).astype(np.float32)
        s = np.float32(np.sum(diag, dtype=np.float32))
        w = diag / s
        residual = (
            np.asarray(target_batch, dtype=np.float32)
            - np.asarray(pred_batch, dtype=np.float32)
        )[:, 0]
        loss = np.float32(np.sum(w * residual * residual, dtype=np.float32))
    return np.asarray(loss, dtype=np.float32)
